# revision 1
# baseline (speedup 1.0000x reference)
"""DiagonalLSTM Bass/Tile kernel for TRN2 (per-core shard: B=4 images).

Layout ("DESIGN-C", row-parity packed):
  State rows p2 (0..63) are split by parity u = p2 % 2 into two halves that
  live on partition halves [64u:64u+64] of gate-space tiles, or in two
  separate rhs tiles A2_u for the matmuls.

  - A2_u  [128,128] SBUF: parts 0:64  = h[k, (b, kap)]  (p = 2*kap + u)
                          parts 64:128= x_t[c, (b, kap)] (skewed input col,
                          zero outside the diagonal band)
  - C2    [128,128] SBUF: C2[64u+k, b*32 + kap] = c[k, (b, p2=2*kap+u)]
  - P01/P23 [128,256] PSUM: gate preactivations, partition 64sigma+m for
    s-pair blocks; cols (b, p) plain.
  - GATES [128,512] SBUF: [64u+k, q*128 + b*32 + 2*mt + w] =
    sigmoid(...)(gate q, p2 = 4*mt + u + 2*w, k)  -- the model's quirky
    flat-split maps quarter q to skew-rows p = 16q+mt and channel o = 64s+k.

Per step t (0..126): 14 small matmuls (s2s+i2s K-packed, s-pair M-packed,
parity-split), 2 sigmoids, full-lane DVE gate math, upsample matmul into
the in-band diagonal of the output buffer. Everything stays on-chip; DRAM
is touched only for the initial input load and final output store.
"""
from contextlib import ExitStack

import numpy as np

import concourse.bass as bass
import concourse.tile as tile
from concourse import bacc, mybir

F32 = mybir.dt.float32
AF = mybir.ActivationFunctionType
ALU = mybir.AluOpType

B = 4          # images per core
H = 64         # rows
W = 64         # cols
C = 64         # input channels
HID = 64       # hidden
NW = H + W - 1 # 127 diagonal steps


def v(ap, off, dims):
    """Custom view: keep ap's partition dim, replace free dims, add offset
    (in elements)."""
    return bass.AP(ap.tensor, ap.offset + off, [list(ap.ap[0])] + [list(d) for d in dims])


def dv(ap, off, dims):
    """Fully-custom view (DRAM side of DMAs): absolute offset, all dims."""
    return bass.AP(ap.tensor, off, [list(d) for d in dims])


def band(t):
    return max(0, t - (W - 1)), min(H - 1, t)


def parity_band(t, u):
    """(kap0, n) for rows p in band(t) with p % 2 == u; n may be 0."""
    lo, hi = band(t)
    p0 = lo + ((u - lo) % 2)
    if p0 > hi:
        return 0, 0
    return (p0 - u) // 2, (hi - p0) // 2 + 1


def build_kernel(ctx, tc, outs, ins):
    nc = tc.nc
    x_d = ins["inputs"]
    out_d = outs["out"]

    const = ctx.enter_context(tc.tile_pool(name="const", bufs=1))
    big = ctx.enter_context(tc.tile_pool(name="big", bufs=1))
    st = ctx.enter_context(tc.tile_pool(name="st", bufs=2))
    tmp = ctx.enter_context(tc.tile_pool(name="tmp", bufs=2))
    ps = ctx.enter_context(tc.tile_pool(name="ps", bufs=2, space="PSUM"))

    # ---------------- weights / biases (one-time prep) ----------------
    # lhsT layouts; matmul computes lhsT.T @ rhs.
    LA01 = const.tile([128, 128], F32, tag="LA01")  # [[Ws1 o=0:128].T ; [Wi2s o=0:128].T]
    LA23 = const.tile([128, 128], F32, tag="LA23")
    LB01 = const.tile([64, 128], F32, tag="LB01")   # Ws0[0:128].T
    LB23 = const.tile([64, 128], F32, tag="LB23")
    LC1 = const.tile([64, 64], F32, tag="LC1")      # Wc1.T
    LC0 = const.tile([64, 64], F32, tag="LC0")
    LU = const.tile([64, 128], F32, tag="LU")       # w_up.T
    bi2s = const.tile([128, 2], F32, tag="bi2s")    # col 0: b_i2s, col 1: b_s2s
    bsg01 = const.tile([128, 1], F32, tag="bsg01")
    bi2s_b = const.tile([128, 2], F32, tag="bi2s_b")
    bsg23 = const.tile([128, 1], F32, tag="bsg23")
    bc2c2 = const.tile([128, 1], F32, tag="bc2c2")
    bup = const.tile([128, 1], F32, tag="bup")

    w_s2s = ins["w_s2s"]   # [256, 64, 2] dram
    w_i2s = ins["w_i2s"]   # [256, 64]
    w_c2c = ins["w_c2c"]   # [64, 64, 2]
    w_up = ins["w_up"]     # [128, 64]

    for blk, LA, LB in ((0, LA01, LB01), (1, LA23, LB23)):
        # LA[kk,m] = Ws1[128*blk+m, kk] (kk<64) | Wi2s[128*blk+m, kk-64]
        nc.sync.dma_start(
            out=LA[0:64, :],
            in_=dv(w_s2s, 128 * blk * 128 + 1, [[2, 64], [128, 128]]),
        )
        nc.sync.dma_start(
            out=LA[64:128, :],
            in_=dv(w_i2s, 128 * blk * 64, [[1, 64], [64, 128]]),
        )
        nc.sync.dma_start(
            out=LB[:, :],
            in_=dv(w_s2s, 128 * blk * 128 + 0, [[2, 64], [128, 128]]),
        )
    nc.sync.dma_start(out=LC1[:, :], in_=dv(w_c2c, 1, [[2, 64], [128, 64]]))
    nc.sync.dma_start(out=LC0[:, :], in_=dv(w_c2c, 0, [[2, 64], [128, 64]]))
    nc.sync.dma_start(out=LU[:, :], in_=dv(w_up, 0, [[1, 64], [64, 128]]))

    b_i2s, b_s2s, b_c2c, b_up = ins["b_i2s"], ins["b_s2s"], ins["b_c2c"], ins["b_up"]
    for blk, (btile, bout) in ((0, (bi2s, bsg01)), (1, (bi2s_b, bsg23))):
        nc.sync.dma_start(out=btile[:, 0:1], in_=dv(b_i2s, 128 * blk, [[1, 128], [1, 1]]))
        nc.sync.dma_start(out=btile[:, 1:2], in_=dv(b_s2s, 128 * blk, [[1, 128], [1, 1]]))
        nc.vector.tensor_add(bout[:, :], btile[:, 0:1], btile[:, 1:2])
    nc.sync.dma_start(out=bc2c2[0:64, :], in_=dv(b_c2c, 0, [[1, 64], [1, 1]]))
    nc.sync.dma_start(out=bc2c2[64:128, :], in_=dv(b_c2c, 0, [[1, 64], [1, 1]]))
    nc.sync.dma_start(out=bup[:, :], in_=dv(b_up, 0, [[1, 128], [1, 1]]))

    # ---------------- input load ----------------
    # IN[c, b*4096 + p*64 + w] = inputs[b, c, p, w]
    IN = big.tile([64, B * H * W], F32, tag="IN")
    for b in range(B):
        nc.sync.dma_start(
            out=IN[:, b * H * W:(b + 1) * H * W],
            in_=dv(x_d, b * C * H * W, [[4096, 64], [1, 4096]]),
        )

    OUT = big.tile([128, B * H * W], F32, tag="OUT")
    IN_ap = IN[:, :]
    OUT_ap = OUT[:, :]

    # ---------------- initial state ----------------
    def xprep(A2_pair, t):
        """Fill x halves of A2_pair (list of 2 tiles) for step t."""
        for u in (0, 1):
            xa = A2_pair[u][64:128, :]
            nc.gpsimd.memset(xa, 0.0)
            k0, n = parity_band(t, u)
            if n:
                nc.gpsimd.tensor_copy(
                    out=v(xa, k0, [[32, 4], [1, n]]),
                    in_=v(IN_ap, 63 * (2 * k0 + u) + t, [[4096, 4], [126, n]]),
                )

    A2 = [st.tile([128, 128], F32, tag=f"A2{u}", name=f"A2{u}") for u in (0, 1)]
    nc.gpsimd.memset(A2[0][0:64, :], 0.0)
    nc.gpsimd.memset(A2[1][0:64, :], 0.0)
    xprep(A2, 0)
    C2 = st.tile([128, 128], F32, tag="C2")
    nc.gpsimd.memset(C2[:, :], 0.0)
    # odd half of c-state, re-based to partition 0 (matmul rhs must share
    # the lhsT's base partition)
    C2o = st.tile([64, 128], F32, tag="C2o")
    nc.gpsimd.memset(C2o[:, :], 0.0)

    # ---------------- the recurrence ----------------
    for t in range(NW):
        P01 = ps.tile([128, 256], F32, tag="P01")
        P23 = ps.tile([128, 256], F32, tag="P23")
        Cp = ps.tile([128, 128], F32, tag="Cp")
        U = ps.tile([128, 256], F32, tag="U")

        # s2s + i2s: for each s-pair block (P01, P23) and each parity u:
        #   out cols p === u (mod 2)
        for P, LA, LB in ((P01, LA01, LB01), (P23, LA23, LB23)):
            Pap = P[:, :]
            for u in (0, 1):
                nc.tensor.matmul(
                    v(Pap, u, [[64, 4], [2, 32]]),
                    LA[:, :], A2[u][:, :],
                    start=(u == 0), stop=False,
                )
            # shift taps: out p === 1 <- h[p-1] (parity 0), full
            nc.tensor.matmul(
                v(Pap, 1, [[64, 4], [2, 32]]),
                LB[:, :], A2[0][0:64, :],
                start=False, stop=False,
            )
            # out p === 0, p >= 2 <- h[p-1] (parity 1), kap' = kap-1
            nc.tensor.matmul(
                v(Pap, 2, [[64, 4], [2, 31]]),
                LB[:, :], v(A2[1][0:64, :], 0, [[32, 4], [1, 31]]),
                start=False, stop=True,
            )

        # c2c: Cp[64u+k, (b,kap)] = Wc1 @ c[p2] + Wc0 @ c[p2-1]
        # (PSUM accumulation groups are tracked per partition: one
        # start/stop pair per partition half)
        nc.tensor.matmul(Cp[0:64, :], LC1[:, :], C2[0:64, :], start=True, stop=False,
                         skip_group_check=True)
        nc.tensor.matmul(Cp[64:128, :], LC1[:, :], C2o[:, :], start=True, stop=False,
                         skip_group_check=True)
        # u=1 out += Wc0 @ c-even (same kap)
        nc.tensor.matmul(Cp[64:128, :], LC0[:, :], C2[0:64, :], start=False, stop=True,
                         skip_group_check=True)
        # u=0 out (kap>=1) += Wc0 @ c-odd (kap-1)
        nc.tensor.matmul(
            v(Cp[0:64, :], 1, [[32, 4], [1, 31]]),
            LC0[:, :], v(C2o[:, :], 0, [[32, 4], [1, 31]]),
            start=False, stop=True, skip_group_check=True,
        )

        # sigmoids: P -> GATES scatter
        G = tmp.tile([128, 512], F32, tag="G")
        Gap = G[:, :]
        for P, bsg, w in ((P01, bsg01, 0), (P23, bsg23, 1)):
            nc.scalar.activation(
                v(Gap, w, [[128, 4], [32, 4], [2, 16]]),
                v(P[:, :], 0, [[16, 4], [64, 4], [1, 16]]),
                AF.Sigmoid, bias=bsg[:, 0:1],
            )

        # gate math (full-lane [128,128])
        T1 = tmp.tile([128, 128], F32, tag="T1")
        T2 = tmp.tile([128, 128], F32, tag="T2")
        nc.vector.tensor_mul(T1[:, :], G[:, 0:128], G[:, 128:256])
        nc.vector.scalar_tensor_tensor(
            out=T2[:, :], in0=Cp[:, :], scalar=bc2c2[:, 0:1], in1=G[:, 256:384],
            op0=ALU.add, op1=ALU.mult,
        )
        C2n = st.tile([128, 128], F32, tag="C2")
        nc.vector.tensor_add(C2n[:, :], T1[:, :], T2[:, :])
        TH = tmp.tile([128, 128], F32, tag="TH")
        nc.scalar.activation(TH[:, :], C2n[:, :], AF.Tanh)
        C2on = st.tile([64, 128], F32, tag="C2o")
        nc.vector.tensor_copy(C2on[:, :], C2n[64:128, :])

        A2n = [st.tile([128, 128], F32, tag=f"A2{u}", name=f"A2n{u}") for u in (0, 1)]
        for u in (0, 1):
            nc.vector.tensor_mul(
                A2n[u][0:64, :], G[64 * u:64 * u + 64, 384:512],
                TH[64 * u:64 * u + 64, :],
            )

        # upsample matmul on the in-band part of h_t, + bias, into OUT diag
        Uap = U[:, :]
        started = False
        for u in (0, 1):
            k0, n = parity_band(t, u)
            if not n:
                continue
            nc.tensor.matmul(
                v(Uap, u * 128 + k0, [[32, 4], [1, n]]),
                LU[:, :],
                v(A2n[u][0:64, :], k0, [[32, 4], [1, n]]),
                start=not started, stop=(u == 1 or parity_band(t, 1)[1] == 0),
            )
            started = True
        for u in (0, 1):
            k0, n = parity_band(t, u)
            if not n:
                continue
            nc.vector.tensor_scalar_add(
                v(OUT_ap, 63 * (2 * k0 + u) + t, [[4096, 4], [126, n]]),
                v(Uap, u * 128 + k0, [[32, 4], [1, n]]),
                bup[:, 0:1],
            )

        # x for next step
        if t + 1 < NW:
            xprep(A2n, t + 1)

        A2 = A2n
        C2 = C2n
        C2o = C2on

    # ---------------- output store ----------------
    # OUT[o2, b*4096 + p*64 + w] -> out[b, o2, p, w]
    for b in range(B):
        nc.sync.dma_start(
            out=dv(out_d, b * 128 * H * W, [[4096, 128], [1, 4096]]),
            in_=OUT[:, b * H * W:(b + 1) * H * W],
        )


def build_nc():
    nc = bacc.Bacc("TRN2", target_bir_lowering=False, debug=False)
    ins = {
        "inputs": nc.dram_tensor("inputs", [B, C, H, W], F32, kind="ExternalInput").ap(),
        "w_i2s": nc.dram_tensor("w_i2s", [4 * HID, C], F32, kind="ExternalInput").ap(),
        "b_i2s": nc.dram_tensor("b_i2s", [4 * HID], F32, kind="ExternalInput").ap(),
        "w_s2s": nc.dram_tensor("w_s2s", [4 * HID, HID, 2], F32, kind="ExternalInput").ap(),
        "b_s2s": nc.dram_tensor("b_s2s", [4 * HID], F32, kind="ExternalInput").ap(),
        "w_c2c": nc.dram_tensor("w_c2c", [HID, HID, 2], F32, kind="ExternalInput").ap(),
        "b_c2c": nc.dram_tensor("b_c2c", [HID], F32, kind="ExternalInput").ap(),
        "w_up": nc.dram_tensor("w_up", [2 * HID, HID], F32, kind="ExternalInput").ap(),
        "b_up": nc.dram_tensor("b_up", [2 * HID], F32, kind="ExternalInput").ap(),
    }
    outs = {"out": nc.dram_tensor("out", [B, 2 * HID, H, W], F32, kind="ExternalOutput").ap()}
    with tile.TileContext(nc) as tc:
        with ExitStack() as ctx:
            build_kernel(ctx, tc, outs, ins)
    nc.compile()
    return nc


# ---------------------------------------------------------------------------
# Harness entry point: full inputs -> shard over 8 cores -> full output.
# ---------------------------------------------------------------------------
from concourse.bass_utils import run_bass_kernel_spmd

N_CORES = 8
TRACE = False
LAST_EXEC_NS = None
_NC = None


def _get_nc():
    global _NC
    if _NC is None:
        _NC = build_nc()
    return _NC


def kernel(**inputs):
    global LAST_EXEC_NS
    nc = _get_nc()
    full = {k: np.ascontiguousarray(np.asarray(val, np.float32))
            for k, val in inputs.items()}
    in_maps = []
    for i in range(N_CORES):
        m = dict(full)
        m["inputs"] = np.ascontiguousarray(full["inputs"][B * i:B * (i + 1)])
        in_maps.append(m)
    res = run_bass_kernel_spmd(nc, in_maps, list(range(N_CORES)), trace=TRACE)
    LAST_EXEC_NS = res.exec_time_ns
    return np.concatenate([res.results[i]["out"] for i in range(N_CORES)], axis=0)


# revision 2
# speedup vs baseline: 1.6017x; 1.6017x over previous
"""DiagonalLSTM Bass/Tile kernel for TRN2 (per-core shard: B=4 images).

Layout ("DESIGN-C", row-parity packed):
  State rows p2 (0..63) are split by parity u = p2 % 2 into two halves that
  live on partition halves [64u:64u+64] of gate-space tiles, or in two
  separate rhs tiles A2_u for the matmuls.

  - A2_u  [128,128] SBUF: parts 0:64  = h[k, (b, kap)]  (p = 2*kap + u)
                          parts 64:128= x_t[c, (b, kap)] (skewed input col,
                          zero outside the diagonal band)
  - C2    [128,128] SBUF: C2[64u+k, b*32 + kap] = c[k, (b, p2=2*kap+u)]
  - P01/P23 [128,256] PSUM: gate preactivations, partition 64sigma+m for
    s-pair blocks; cols (b, p) plain.
  - GATES [128,512] SBUF: [64u+k, q*128 + b*32 + 2*mt + w] =
    sigmoid(...)(gate q, p2 = 4*mt + u + 2*w, k)  -- the model's quirky
    flat-split maps quarter q to skew-rows p = 16q+mt and channel o = 64s+k.

Per step t (0..126): 14 small matmuls (s2s+i2s K-packed, s-pair M-packed,
parity-split), 2 sigmoids, full-lane DVE gate math, upsample matmul into
the in-band diagonal of the output buffer. Everything stays on-chip; DRAM
is touched only for the initial input load and final output store.
"""
from contextlib import ExitStack

import numpy as np

import concourse.bass as bass
import concourse.tile as tile
from concourse import bacc, mybir

F32 = mybir.dt.float32
BF = mybir.dt.bfloat16
AF = mybir.ActivationFunctionType
ALU = mybir.AluOpType

B = 4          # images per core
H = 64         # rows
W = 64         # cols
C = 64         # input channels
HID = 64       # hidden
NW = H + W - 1 # 127 diagonal steps


def v(ap, off, dims):
    """Custom view: keep ap's partition dim, replace free dims, add offset
    (in elements)."""
    return bass.AP(ap.tensor, ap.offset + off, [list(ap.ap[0])] + [list(d) for d in dims])


def dv(ap, off, dims):
    """Fully-custom view (DRAM side of DMAs): absolute offset, all dims."""
    return bass.AP(ap.tensor, off, [list(d) for d in dims])


def band(t):
    return max(0, t - (W - 1)), min(H - 1, t)


def parity_band(t, u):
    """(kap0, n) for rows p in band(t) with p % 2 == u; n may be 0."""
    lo, hi = band(t)
    p0 = lo + ((u - lo) % 2)
    if p0 > hi:
        return 0, 0
    return (p0 - u) // 2, (hi - p0) // 2 + 1


def build_kernel(ctx, tc, outs, ins):
    nc = tc.nc
    x_d = ins["inputs"]
    out_d = outs["out"]

    const = ctx.enter_context(tc.tile_pool(name="const", bufs=1))
    big = ctx.enter_context(tc.tile_pool(name="big", bufs=1))
    st = ctx.enter_context(tc.tile_pool(name="st", bufs=2))
    tmp = ctx.enter_context(tc.tile_pool(name="tmp", bufs=2))
    ps = ctx.enter_context(tc.tile_pool(name="ps", bufs=2, space="PSUM"))

    # ---------------- weights / biases (one-time prep) ----------------
    # lhsT layouts; matmul computes lhsT.T @ rhs.
    LA01 = const.tile([128, 128], BF, tag="LA01")  # [[Ws1 o=0:128].T ; [Wi2s o=0:128].T]
    LA23 = const.tile([128, 128], BF, tag="LA23")
    LB01 = const.tile([64, 128], BF, tag="LB01")   # Ws0[0:128].T
    LB23 = const.tile([64, 128], BF, tag="LB23")
    LC1 = const.tile([64, 64], BF, tag="LC1")      # Wc1.T
    LC0 = const.tile([64, 64], BF, tag="LC0")
    LU = const.tile([64, 128], BF, tag="LU")       # w_up.T
    LA01f = const.tile([128, 128], F32, tag="LA01f")
    LA23f = const.tile([128, 128], F32, tag="LA23f")
    LB01f = const.tile([64, 128], F32, tag="LB01f")
    LB23f = const.tile([64, 128], F32, tag="LB23f")
    LC1f = const.tile([64, 64], F32, tag="LC1f")
    LC0f = const.tile([64, 64], F32, tag="LC0f")
    LUf = const.tile([64, 128], F32, tag="LUf")
    bi2s = const.tile([128, 2], F32, tag="bi2s")    # col 0: b_i2s, col 1: b_s2s
    bsg01 = const.tile([128, 1], F32, tag="bsg01")
    bi2s_b = const.tile([128, 2], F32, tag="bi2s_b")
    bsg23 = const.tile([128, 1], F32, tag="bsg23")
    bc2c2 = const.tile([128, 1], F32, tag="bc2c2")
    bup = const.tile([128, 1], F32, tag="bup")

    w_s2s = ins["w_s2s"]   # [256, 64, 2] dram
    w_i2s = ins["w_i2s"]   # [256, 64]
    w_c2c = ins["w_c2c"]   # [64, 64, 2]
    w_up = ins["w_up"]     # [128, 64]

    for blk, LA, LB in ((0, LA01f, LB01f), (1, LA23f, LB23f)):
        # LA[kk,m] = Ws1[128*blk+m, kk] (kk<64) | Wi2s[128*blk+m, kk-64]
        nc.sync.dma_start(
            out=LA[0:64, :],
            in_=dv(w_s2s, 128 * blk * 128 + 1, [[2, 64], [128, 128]]),
        )
        nc.sync.dma_start(
            out=LA[64:128, :],
            in_=dv(w_i2s, 128 * blk * 64, [[1, 64], [64, 128]]),
        )
        nc.sync.dma_start(
            out=LB[:, :],
            in_=dv(w_s2s, 128 * blk * 128 + 0, [[2, 64], [128, 128]]),
        )
    nc.sync.dma_start(out=LC1f[:, :], in_=dv(w_c2c, 1, [[2, 64], [128, 64]]))
    nc.sync.dma_start(out=LC0f[:, :], in_=dv(w_c2c, 0, [[2, 64], [128, 64]]))
    nc.sync.dma_start(out=LUf[:, :], in_=dv(w_up, 0, [[1, 64], [64, 128]]))
    for bf_t, f_t in ((LA01, LA01f), (LA23, LA23f), (LB01, LB01f), (LB23, LB23f),
                      (LC1, LC1f), (LC0, LC0f), (LU, LUf)):
        nc.vector.tensor_copy(bf_t[:, :], f_t[:, :])

    b_i2s, b_s2s, b_c2c, b_up = ins["b_i2s"], ins["b_s2s"], ins["b_c2c"], ins["b_up"]
    for blk, (btile, bout) in ((0, (bi2s, bsg01)), (1, (bi2s_b, bsg23))):
        nc.sync.dma_start(out=btile[:, 0:1], in_=dv(b_i2s, 128 * blk, [[1, 128], [1, 1]]))
        nc.sync.dma_start(out=btile[:, 1:2], in_=dv(b_s2s, 128 * blk, [[1, 128], [1, 1]]))
        nc.vector.tensor_add(bout[:, :], btile[:, 0:1], btile[:, 1:2])
    nc.sync.dma_start(out=bc2c2[0:64, :], in_=dv(b_c2c, 0, [[1, 64], [1, 1]]))
    nc.sync.dma_start(out=bc2c2[64:128, :], in_=dv(b_c2c, 0, [[1, 64], [1, 1]]))
    nc.sync.dma_start(out=bup[:, :], in_=dv(b_up, 0, [[1, 128], [1, 1]]))

    # ---------------- input load ----------------
    # IN[c, b*4096 + p*64 + w] = inputs[b, c, p, w]
    IN = big.tile([64, B * H * W], F32, tag="IN")
    for b in range(B):
        nc.sync.dma_start(
            out=IN[:, b * H * W:(b + 1) * H * W],
            in_=dv(x_d, b * C * H * W, [[4096, 64], [1, 4096]]),
        )

    OUT = big.tile([128, B * H * W], F32, tag="OUT")
    IN_ap = IN[:, :]
    OUT_ap = OUT[:, :]

    # ---------------- initial state ----------------
    def xprep(A2_pair, t):
        """Fill x halves of A2_pair (list of 2 tiles) for step t."""
        for u in (0, 1):
            xa = A2_pair[u][64:128, :]
            nc.gpsimd.memset(xa, 0.0)
            k0, n = parity_band(t, u)
            if n:
                nc.gpsimd.tensor_copy(
                    out=v(xa, k0, [[32, 4], [1, n]]),
                    in_=v(IN_ap, 63 * (2 * k0 + u) + t, [[4096, 4], [126, n]]),
                )

    A2 = [st.tile([128, 128], BF, tag=f"A2{u}", name=f"A2{u}") for u in (0, 1)]
    nc.gpsimd.memset(A2[0][0:64, :], 0.0)
    nc.gpsimd.memset(A2[1][0:64, :], 0.0)
    xprep(A2, 0)
    # bf16 halves of c-state (matmul rhs; both re-based to partition 0 --
    # matmul rhs must share the lhsT's base partition)
    C2e = st.tile([64, 128], BF, tag="C2e")
    nc.gpsimd.memset(C2e[:, :], 0.0)
    C2o = st.tile([64, 128], BF, tag="C2o")
    nc.gpsimd.memset(C2o[:, :], 0.0)

    # ---------------- the recurrence ----------------
    for t in range(NW):
        P01 = ps.tile([128, 256], F32, tag="P01")
        P23 = ps.tile([128, 256], F32, tag="P23")
        Cp = ps.tile([128, 128], F32, tag="Cp")
        U = ps.tile([128, 256], F32, tag="U")

        # s2s + i2s: for each s-pair block (P01, P23) and each parity u:
        #   out cols p === u (mod 2)
        for P, LA, LB in ((P01, LA01, LB01), (P23, LA23, LB23)):
            Pap = P[:, :]
            for u in (0, 1):
                nc.tensor.matmul(
                    v(Pap, u, [[64, 4], [2, 32]]),
                    LA[:, :], A2[u][:, :],
                    start=(u == 0), stop=False,
                )
            # shift taps: out p === 1 <- h[p-1] (parity 0), full
            nc.tensor.matmul(
                v(Pap, 1, [[64, 4], [2, 32]]),
                LB[:, :], A2[0][0:64, :],
                start=False, stop=False,
            )
            # out p === 0, p >= 2 <- h[p-1] (parity 1), kap' = kap-1
            nc.tensor.matmul(
                v(Pap, 2, [[64, 4], [2, 31]]),
                LB[:, :], v(A2[1][0:64, :], 0, [[32, 4], [1, 31]]),
                start=False, stop=True,
            )

        # c2c: Cp[64u+k, (b,kap)] = Wc1 @ c[p2] + Wc0 @ c[p2-1]
        # (PSUM accumulation groups are tracked per partition: one
        # start/stop pair per partition half)
        nc.tensor.matmul(Cp[0:64, :], LC1[:, :], C2e[:, :], start=True, stop=False,
                         skip_group_check=True)
        nc.tensor.matmul(Cp[64:128, :], LC1[:, :], C2o[:, :], start=True, stop=False,
                         skip_group_check=True)
        # u=1 out += Wc0 @ c-even (same kap)
        nc.tensor.matmul(Cp[64:128, :], LC0[:, :], C2e[:, :], start=False, stop=True,
                         skip_group_check=True)
        # u=0 out (kap>=1) += Wc0 @ c-odd (kap-1)
        nc.tensor.matmul(
            v(Cp[0:64, :], 1, [[32, 4], [1, 31]]),
            LC0[:, :], v(C2o[:, :], 0, [[32, 4], [1, 31]]),
            start=False, stop=True, skip_group_check=True,
        )

        # sigmoids: P -> GATES scatter
        G = tmp.tile([128, 512], F32, tag="G")
        Gap = G[:, :]
        for P, bsg, w in ((P01, bsg01, 0), (P23, bsg23, 1)):
            nc.scalar.activation(
                v(Gap, w, [[128, 4], [32, 4], [2, 16]]),
                v(P[:, :], 0, [[16, 4], [64, 4], [1, 16]]),
                AF.Sigmoid, bias=bsg[:, 0:1],
            )

        # gate math (full-lane [128,128])
        T1 = tmp.tile([128, 128], F32, tag="T1")
        T2 = tmp.tile([128, 128], F32, tag="T2")
        nc.vector.tensor_mul(T1[:, :], G[:, 0:128], G[:, 128:256])
        nc.vector.scalar_tensor_tensor(
            out=T2[:, :], in0=Cp[:, :], scalar=bc2c2[:, 0:1], in1=G[:, 256:384],
            op0=ALU.add, op1=ALU.mult,
        )
        C2n = tmp.tile([128, 128], F32, tag="C2n")
        nc.vector.tensor_add(C2n[:, :], T1[:, :], T2[:, :])
        TH = tmp.tile([128, 128], F32, tag="TH")
        nc.scalar.activation(TH[:, :], C2n[:, :], AF.Tanh)
        C2en = st.tile([64, 128], BF, tag="C2e")
        nc.vector.tensor_copy(C2en[:, :], C2n[0:64, :])
        C2on = st.tile([64, 128], BF, tag="C2o")
        nc.vector.tensor_copy(C2on[:, :], C2n[64:128, :])

        A2n = [st.tile([128, 128], BF, tag=f"A2{u}", name=f"A2n{u}") for u in (0, 1)]
        for u in (0, 1):
            nc.vector.tensor_mul(
                A2n[u][0:64, :], G[64 * u:64 * u + 64, 384:512],
                TH[64 * u:64 * u + 64, :],
            )

        # upsample matmul on the in-band part of h_t, + bias, into OUT diag
        Uap = U[:, :]
        started = False
        for u in (0, 1):
            k0, n = parity_band(t, u)
            if not n:
                continue
            nc.tensor.matmul(
                v(Uap, u * 128 + k0, [[32, 4], [1, n]]),
                LU[:, :],
                v(A2n[u][0:64, :], k0, [[32, 4], [1, n]]),
                start=not started, stop=(u == 1 or parity_band(t, 1)[1] == 0),
            )
            started = True
        for u in (0, 1):
            k0, n = parity_band(t, u)
            if not n:
                continue
            nc.vector.tensor_scalar_add(
                v(OUT_ap, 63 * (2 * k0 + u) + t, [[4096, 4], [126, n]]),
                v(Uap, u * 128 + k0, [[32, 4], [1, n]]),
                bup[:, 0:1],
            )

        # x for next step
        if t + 1 < NW:
            xprep(A2n, t + 1)

        A2 = A2n
        C2e = C2en
        C2o = C2on

    # ---------------- output store ----------------
    # OUT[o2, b*4096 + p*64 + w] -> out[b, o2, p, w]
    for b in range(B):
        nc.sync.dma_start(
            out=dv(out_d, b * 128 * H * W, [[4096, 128], [1, 4096]]),
            in_=OUT[:, b * H * W:(b + 1) * H * W],
        )


def build_nc():
    nc = bacc.Bacc("TRN2", target_bir_lowering=False, debug=False)
    ins = {
        "inputs": nc.dram_tensor("inputs", [B, C, H, W], F32, kind="ExternalInput").ap(),
        "w_i2s": nc.dram_tensor("w_i2s", [4 * HID, C], F32, kind="ExternalInput").ap(),
        "b_i2s": nc.dram_tensor("b_i2s", [4 * HID], F32, kind="ExternalInput").ap(),
        "w_s2s": nc.dram_tensor("w_s2s", [4 * HID, HID, 2], F32, kind="ExternalInput").ap(),
        "b_s2s": nc.dram_tensor("b_s2s", [4 * HID], F32, kind="ExternalInput").ap(),
        "w_c2c": nc.dram_tensor("w_c2c", [HID, HID, 2], F32, kind="ExternalInput").ap(),
        "b_c2c": nc.dram_tensor("b_c2c", [HID], F32, kind="ExternalInput").ap(),
        "w_up": nc.dram_tensor("w_up", [2 * HID, HID], F32, kind="ExternalInput").ap(),
        "b_up": nc.dram_tensor("b_up", [2 * HID], F32, kind="ExternalInput").ap(),
    }
    outs = {"out": nc.dram_tensor("out", [B, 2 * HID, H, W], F32, kind="ExternalOutput").ap()}
    with tile.TileContext(nc) as tc:
        with ExitStack() as ctx:
            build_kernel(ctx, tc, outs, ins)
    nc.compile()
    return nc


# ---------------------------------------------------------------------------
# Harness entry point: full inputs -> shard over 8 cores -> full output.
# ---------------------------------------------------------------------------
from concourse.bass_utils import run_bass_kernel_spmd

N_CORES = 8
TRACE = False
LAST_EXEC_NS = None
_NC = None


def _get_nc():
    global _NC
    if _NC is None:
        _NC = build_nc()
    return _NC


def kernel(**inputs):
    global LAST_EXEC_NS
    nc = _get_nc()
    full = {k: np.ascontiguousarray(np.asarray(val, np.float32))
            for k, val in inputs.items()}
    in_maps = []
    for i in range(N_CORES):
        m = dict(full)
        m["inputs"] = np.ascontiguousarray(full["inputs"][B * i:B * (i + 1)])
        in_maps.append(m)
    res = run_bass_kernel_spmd(nc, in_maps, list(range(N_CORES)), trace=TRACE)
    LAST_EXEC_NS = res.exec_time_ns
    return np.concatenate([res.results[i]["out"] for i in range(N_CORES)], axis=0)
